# revision 45
# baseline (speedup 1.0000x reference)
"""AttentionBlock (GroupNorm -> 1x1-conv QKV -> softmax attention -> 1x1-conv proj
-> residual) for Trainium2, data-parallel over batch across 8 NeuronCores.

Shapes (hardcoded): x [B=8, C=64, H=64, W=64] fp32; N = H*W = 4096.
Each core processes one sample end-to-end; no cross-core communication.

Key Trainium facts that shape this kernel:
  - A matmul with contraction K<=64 streams at HALF rate (64-row tiling mode);
    K=128 streams 1 column/cycle. With C=64 channels, all hot matmuls are
    made K=128 by duplicating operands on both partition halves and halving
    the stacked weights (sum over 128 partitions of duplicated data = 2x).
  - fp32 matmuls run as two PE passes and their self-loading LDWEIGHTS only
    supports one sync wait; bf16 is one pass (and scores are O(1), so bf16
    keeps ~3 digits -> final error ~1e-4).
  - ScalarE exp runs at 1 elem/lane/cycle -> 16.7M exps/core ~ 115us is the
    roofline engine; everything else is arranged to hide under it.

Per-core pipeline:
  1. GroupNorm: per-channel bn_stats/bn_aggr on x2x[0:64] -> tiny mask
     matmuls reduce/broadcast the 8-channel groups -> one fused affine
     produces h2x [128, N] bf16 (h duplicated on both partition halves).
  2. q2x = (Wq h + bq)/16 and k2x = Wk h, both [128, N] bf16 duplicated
     (bk dropped: constant shift per softmax row). vT [N, C+1] bf16 with a
     ones column so the AV matmul also accumulates the softmax denominator.
  3. sT[m, n] tiles = k2x.T @ q2x (K=128), exp on ScalarE PSUM->SBUF (score
     range is ~[-3, 3]: no row-max subtraction needed), AV accumulates
     out[c, n] + den[n] over the 32 m-chunks.
  4. proj = Wp @ out_unnormalized, scaled by 1/den (column scaling commutes
     with the left matmul; reciprocal via a DMA partition-broadcast of den
     and the fast DVE approx reciprocal), + (bp + Wp bv) + residual x.

The nt loop is software-pipelined (scores/exp of tile nt interleaved with AV
of tile nt-1) so the PE stream stays dense and ScalarE never starves.
"""

import os
import numpy as np
import ml_dtypes

import concourse.bass as bass
import concourse.bacc as bacc
import concourse.mybir as mybir
from concourse.tile import TileContext
from concourse.bass_utils import run_bass_kernel_spmd

FP = mybir.dt.float32
F16 = mybir.dt.bfloat16
B, C, H, W = 8, 64, 64, 64
N = H * W          # 4096
G = 8              # groups
NT = 512           # n-tile (free dim of score tiles)
MT = 128           # m-tile (partition dim of score tiles)
N_NT = N // NT     # 8
N_MT = N // MT     # 32
NPAIR = N_MT // 2  # 16 score psum groups (2 m-chunks each) per n-tile
EPS = 1e-5
COPY = mybir.ActivationFunctionType.Copy

last_run_info = {}


class OneActSetBacc(bacc.Bacc):
    """All ACT functions used here (exp, ln, square, copy) live in the
    natural_log_exp_and_others table set (id 6). The default per-function
    set choice inserts three ~1.3us table loads on the critical path; force
    every load to set 6 and drop the redundant reloads."""

    NL_EXP_SET = 6

    def insert_act_table_loads(self):
        super().insert_act_table_loads()
        for blk in self.main_func.blocks:
            keep = []
            seen = False
            for ins in blk.instructions:
                if isinstance(ins, mybir.InstLoadActFuncSet):
                    ins.act_func_set_id = self.NL_EXP_SET
                    si = ins.sync_info
                    clean = si is None or (not si.on_wait and not si.on_update)
                    if seen and clean:
                        continue
                    seen = True
                keep.append(ins)
            if len(keep) != len(blk.instructions):
                blk.instructions[:] = keep


def build_program(debug=False):
    # Bacc (not raw Bass): its finalize pipeline splits multi-sem waits.
    nc = OneActSetBacc()
    dbg = {}
    if debug:
        for nm, shp in [("dbg_h", [128, N]), ("dbg_q", [128, N]), ("dbg_k", [128, N]),
                        ("dbg_vt", [128, N_MT * (C + 1)]),
                        ("dbg_av", [C, N]), ("dbg_den", [1, N])]:
            dbg[nm] = nc.dram_tensor(nm, shp, FP, kind="ExternalOutput")

    x_d = nc.dram_tensor("x", [C, N], FP, kind="ExternalInput")
    # All small constants packed into two tensors (one DMA each):
    # cf32 [128, 140]: 0 bq2 | 1 bpp | 2 gamma2 | 3 beta2 | 4:12 gmask | 12:140 gbcast2(rows 0:8)
    # cb16 [128, 448]: 0:128 wq_st | 128:256 wk_st | 256:320 wv_st | 320:384 wpT | 384:448 wpwvT
    cf32_d = nc.dram_tensor("cf32", [128, 140], FP, kind="ExternalInput")
    cb16_d = nc.dram_tensor("cb16", [128, 448], F16, kind="ExternalInput")
    out_d = nc.dram_tensor("out", [C, N], FP, kind="ExternalOutput")

    with TileContext(nc) as tc:
        with (
            tc.tile_pool(name="const", bufs=1) as const,
            tc.tile_pool(name="big", bufs=1) as big,
            tc.tile_pool(name="epool", bufs=2) as epool,
            tc.tile_pool(name="small", bufs=4) as small,
            tc.tile_pool(name="outp", bufs=3) as outp,
            tc.tile_pool(name="dram", bufs=2, space="DRAM") as drampool,
            tc.tile_pool(name="qk_ps", bufs=2, space="PSUM") as qk_ps,
            tc.tile_pool(name="av_ps", bufs=2, space="PSUM") as av_ps,
            tc.tile_pool(name="post_ps", bufs=2, space="PSUM") as post_ps,
        ):
            # ---- constant loads (2 packed DMAs; DVE-funneled because a
            # matmul's self-loading LDWEIGHTS supports only one sync wait,
            # so matmul operands must not depend directly on DMA) ----
            cf32s = small.tile([128, 140], FP, tag="cf32s")
            cb16s = small.tile([128, 448], F16, tag="cb16s")
            nc.sync.dma_start(out=cf32s[:], in_=cf32_d[:])
            nc.sync.dma_start(out=cb16s[:], in_=cb16_d[:])
            cf32 = const.tile([128, 140], FP, tag="cf32")
            cb16 = const.tile([128, 448], F16, tag="cb16")
            nc.vector.tensor_copy(out=cf32[:], in_=cf32s[:])
            nc.vector.tensor_copy(out=cb16[:], in_=cb16s[:])
            bq2 = cf32[:, 0:1]
            bpp = cf32[0:C, 1:2]
            gamma2 = cf32[:, 2:3]
            beta2 = cf32[:, 3:4]
            gmask = cf32[0:C, 4:12]
            gbcast2 = cf32[0:G, 12:140]
            wq_st = cb16[:, 0:128]
            wk_st = cb16[:, 128:256]
            wv_st = cb16[:, 256:320]
            wpT = cb16[0:C, 320:384]
            wpwvT = cb16[0:C, 384:448]

            eps_sb = const.tile([128, 1], FP, tag="eps")
            nc.vector.memset(eps_sb[:], EPS)
            ones_col = const.tile([128, C], F16, tag="ones_col")
            nc.vector.memset(ones_col[:], 1.0)

            # ---- load x (partitions 0-63 only), stats + bf16 cast ----
            # chunked so stats/cast pipeline with the DMA; the bf16 copy is
            # duplicated onto partitions 64-127 by an SBUF->SBUF DMA (the
            # projections contract over K=128 with half-weights).
            x_sb = big.tile([C, N], FP, tag="x_sb")
            x16 = big.tile([128, N], F16, tag="x16")
            sums = small.tile([C, 2, 2], FP, tag="gn_sums")
            sq_scr = small.tile([C, N // 2], FP, tag="gn_sq_scr")
            for j in range(2):
                sl = slice(j * (N // 2), (j + 1) * (N // 2))
                nc.sync.dma_start(out=x_sb[:, sl], in_=x_d[:, sl])
                nc.scalar.activation(out=sq_scr[:], in_=x_sb[:, sl],
                                     func=mybir.ActivationFunctionType.Square,
                                     accum_out=sums[:, j, 1:2])
                nc.vector.tensor_reduce(op=mybir.AluOpType.add, out=sums[:, j, 0:1],
                                        in_=x_sb[:, sl], axis=mybir.AxisListType.X)
                nc.vector.tensor_copy(out=x16[0:C, sl], in_=x_sb[:, sl])
                nc.sync.dma_start(out=x16[C:128, sl], in_=x16[0:C, sl])
            mm2 = small.tile([C, 2], FP, tag="gn_mm2")
            nc.vector.tensor_add(out=sums[:, 0, :], in0=sums[:, 0, :], in1=sums[:, 1, :])
            nc.vector.tensor_scalar_mul(out=mm2[:], in0=sums[:, 0, :], scalar1=1.0 / N)
            # group stats: [G, 2] = gmask.T @ mm2   (gmask holds 1/8)
            gstat_ps = post_ps.tile([128, 512], FP, tag="post")
            nc.tensor.matmul(out=gstat_ps[0:G, 0:2], lhsT=gmask, rhs=mm2[:])
            gstat = small.tile([G, 2], FP, tag="gn_gstat")
            nc.vector.tensor_copy(out=gstat[:], in_=gstat_ps[0:G, 0:2])
            # var_g = E[x^2]_g - mean_g^2 ; rstd = 1/sqrt(var+eps)
            vg = small.tile([G, 1], FP, tag="gn_vg")
            nc.vector.tensor_mul(out=vg[:], in0=gstat[:, 0:1], in1=gstat[:, 0:1])
            nc.vector.tensor_sub(out=vg[:], in0=gstat[:, 1:2], in1=vg[:])
            # rstd = exp(-0.5*ln(var+eps)) — Ln and Exp share one ACT table
            # set with the attention exp, avoiding a 2.7us sqrt-table load.
            lnv = small.tile([G, 1], FP, tag="gn_lnv")
            nc.scalar.activation(out=lnv[:], in_=vg[:],
                                 func=mybir.ActivationFunctionType.Ln,
                                 bias=eps_sb[0:G, :])
            rhs2 = small.tile([G, 2], FP, tag="gn_rhs2")
            nc.vector.tensor_copy(out=rhs2[:, 0:1], in_=gstat[:, 0:1])
            nc.scalar.activation(out=rhs2[:, 1:2], in_=lnv[:],
                                 func=mybir.ActivationFunctionType.Exp,
                                 scale=-0.5)
            # broadcast to both channel copies: [128, 2] = gbcast2.T @ rhs2
            pstat_ps = post_ps.tile([128, 512], FP, tag="post")
            nc.tensor.matmul(out=pstat_ps[:, 0:2], lhsT=gbcast2, rhs=rhs2[:])
            a_sb = small.tile([128, 1], FP, tag="gn_a")
            b_sb = small.tile([128, 1], FP, tag="gn_b")
            nc.vector.tensor_mul(out=a_sb[:], in0=pstat_ps[:, 1:2], in1=gamma2[:])
            nc.vector.tensor_mul(out=b_sb[:], in0=pstat_ps[:, 0:1], in1=a_sb[:])
            nc.vector.tensor_sub(out=b_sb[:], in0=beta2[:], in1=b_sb[:])
            # Fold the affine h = a*x + b into the projections:
            #   w*_eff = w*_st * a (per-partition row scale)
            #   q bias += (Wq b)/16 via a tiny matmul; k's b-term shifts every
            #   score in a softmax row by a constant (drop); v's b-term folds
            #   into the final bias as Wp @ Wv @ b (wpwv const, tiny matmul).
            b16 = small.tile([128, 1], F16, tag="gn_b16")
            nc.vector.tensor_copy(out=b16[:], in_=b_sb[:])
            wq_eff = const.tile([128, 128], F16, tag="wq_eff")
            wk_eff = const.tile([128, 128], F16, tag="wk_eff")
            wv_eff = const.tile([128, C], F16, tag="wv_eff")
            nc.vector.tensor_scalar_mul(out=wq_eff[:], in0=wq_st, scalar1=a_sb[:])
            nc.vector.tensor_scalar_mul(out=wk_eff[:], in0=wk_st, scalar1=a_sb[:])
            nc.vector.tensor_scalar_mul(out=wv_eff[:], in0=wv_st, scalar1=a_sb[:])
            bias_ps = post_ps.tile([128, 512], FP, tag="post")
            nc.tensor.matmul(out=bias_ps[:, 0:1], lhsT=wq_st, rhs=b16[:])
            nc.tensor.matmul(out=bias_ps[0:C, 1:2], lhsT=wpwvT, rhs=b16[0:C, :])
            bq_eff = small.tile([128, 1], FP, tag="bq_eff")
            bpp_eff = small.tile([C, 1], FP, tag="bpp_eff")
            nc.vector.tensor_add(out=bq_eff[:], in0=bias_ps[:, 0:1], in1=bq2)
            nc.vector.tensor_add(out=bpp_eff[:], in0=bias_ps[0:C, 1:2], in1=bpp)

            # ---- QKV projections (bf16, K=128) ----
            q2x = big.tile([128, N], F16, tag="q2x")
            k2x = big.tile([128, N], F16, tag="k2x")
            for j in range(N_NT):
                sl = slice(j * NT, (j + 1) * NT)
                qp = qk_ps.tile([128, 2 * NT], FP, tag="qk")
                nc.tensor.matmul(out=qp[:, 0:NT], lhsT=wq_eff[:], rhs=x16[:, sl])
                nc.tensor.matmul(out=qp[:, NT:2 * NT], lhsT=wk_eff[:], rhs=x16[:, sl])
                # q needs a bias add (VectorE); k is a plain copy (ScalarE)
                nc.vector.tensor_scalar_add(out=q2x[:, sl], in0=qp[:, 0:NT], scalar1=bq_eff[:])
                nc.scalar.activation(out=k2x[:, sl], in_=qp[:, NT:2 * NT], func=COPY)

            # vT_aug[p, mt, 0:64] = v[m = mt*128+p, c]; vT_aug[p, mt, 64] = 1
            vT = big.tile([128, N_MT, C + 1], F16, tag="vT")
            nc.vector.memset(vT[:, :, C:C + 1], 1.0)
            for mt in range(0, N_MT, 4):
                vp = av_ps.tile([128, NT], FP, tag="av")
                for j in range(4):
                    nc.tensor.matmul(out=vp[:, j * C:(j + 1) * C],
                                     lhsT=x16[:, (mt + j) * MT:(mt + j + 1) * MT],
                                     rhs=wv_eff[:])
                nc.scalar.activation(
                    out=vT[:, mt:mt + 4, 0:C],
                    in_=vp[:, 0:4 * C].rearrange("p (j c) -> p j c", j=4),
                    func=COPY)

            if debug:
                dq = big.tile([128, N], FP, tag="dbgq")
                dk = big.tile([128, N], FP, tag="dbgk")
                dv = big.tile([128, N_MT * (C + 1)], FP, tag="dbgv")
                nc.vector.tensor_copy(out=dq[:], in_=q2x[:])
                nc.vector.tensor_copy(out=dk[:], in_=k2x[:])
                nc.vector.tensor_copy(out=dv[:], in_=vT[:].rearrange("p a b -> p (a b)"))
                nc.sync.dma_start(out=dbg["dbg_q"][:], in_=dq[:])
                nc.sync.dma_start(out=dbg["dbg_k"][:], in_=dk[:])
                nc.sync.dma_start(out=dbg["dbg_vt"][:], in_=dv[:])

            # ---- attention (software-pipelined over n-tiles) ----
            e_tiles = {}

            # m-chunk grouping per n-tile: 10 groups of 3 + 1 of 2 so each
            # exp instruction covers [128, 1536] (amortizes ScalarE's
            # per-instruction overhead; 3 PSUM banks per group).
            GROUPS = [(i * 2, 2) for i in range(16)]

            def emit_qk_group(nt, g, e):
                nsl = slice(nt * NT, (nt + 1) * NT)
                mt0, gsz = GROUPS[g]
                sp = qk_ps.tile([128, 2 * NT], FP, tag="qk")
                for j in range(gsz):
                    mt = mt0 + j
                    nc.tensor.matmul(out=sp[:, j * NT:(j + 1) * NT],
                                     lhsT=k2x[:, mt * MT:(mt + 1) * MT],
                                     rhs=q2x[:, nsl])
                nc.scalar.activation(out=e[:, mt0:mt0 + gsz, :],
                                     in_=sp[:, 0:gsz * NT],
                                     func=mybir.ActivationFunctionType.Exp)

            def emit_av_group(av, e, g):
                mt0, gsz = GROUPS[g]
                for j in range(gsz):
                    mt = mt0 + j
                    nc.tensor.matmul(
                        out=av[0:C + 1, :],
                        lhsT=vT[:, mt, :],
                        rhs=e[:, mt, :],
                        start=(mt == 0), stop=(mt == N_MT - 1),
                        skip_group_check=True)

            def emit_post(nt, av):
                nsl = slice(nt * NT, (nt + 1) * NT)
                # den (psum row 64) -> SBUF -> partition-broadcast via DMA
                # (DRAM bounce) -> fast approx reciprocal on 64 partitions.
                den16 = small.tile([128, NT], F16, tag="den16")
                nc.vector.tensor_copy(out=den16[C:C + 1, :], in_=av[C:C + 1, :])
                if debug:
                    den_sb = small.tile([128, NT], FP, tag="den_sb")
                    nc.vector.tensor_copy(out=den_sb[C:C + 1, :], in_=av[C:C + 1, :])
                    nc.sync.dma_start(out=dbg["dbg_den"][:, nsl], in_=den_sb[C:C + 1, :])
                dbc_ps = post_ps.tile([128, 512], FP, tag="post")
                nc.tensor.matmul(out=dbc_ps[0:C, :], lhsT=ones_col[C:C + 1, :],
                                 rhs=den16[C:C + 1, :])
                den_bc = outp.tile([C, NT], FP, tag="den_bc")
                nc.vector.tensor_copy(out=den_bc[:], in_=dbc_ps[0:C, :])
                dbc = outp.tile([C, NT], FP, tag="dbc")
                scr = outp.tile([C, NT], FP, tag="dbc_scr")
                nc.vector.reciprocal_approx_accurate(out=dbc[:], in_=den_bc[:], scratch=scr[:])
                # unnormalized attention output -> SBUF (bf16) for proj matmul
                av_sb = outp.tile([C, NT], F16, tag="av_sb")
                nc.vector.tensor_copy(out=av_sb[:], in_=av[0:C, :])
                if debug:
                    dav = outp.tile([C, NT], FP, tag="dav")
                    nc.vector.tensor_copy(out=dav[:], in_=av[0:C, :])
                    nc.sync.dma_start(out=dbg["dbg_av"][:, nsl], in_=dav[:])
                # proj, then scale columns by 1/den, + bias' + residual
                pj_ps = post_ps.tile([128, 512], FP, tag="post")
                nc.tensor.matmul(out=pj_ps[0:C, :], lhsT=wpT, rhs=av_sb[:])
                o_sb = outp.tile([C, NT], FP, tag="o_sb")
                nc.vector.tensor_mul(out=o_sb[:], in0=pj_ps[0:C, :], in1=dbc[:])
                nc.vector.scalar_tensor_tensor(
                    out=o_sb[:], in0=o_sb[:], scalar=bpp_eff[:], in1=x_sb[:, nsl],
                    op0=mybir.AluOpType.add, op1=mybir.AluOpType.add)
                nc.sync.dma_start(out=out_d[:, nsl], in_=o_sb[:])

            for nt in range(N_NT + 1):
                e_cur = None
                if nt < N_NT:
                    e_cur = epool.tile([128, N_MT, NT], F16, tag="e")
                    e_tiles[nt] = e_cur
                if nt > 0:
                    av_cur = av_ps.tile([128, NT], FP, tag="av", name=f"av_{nt}")
                else:
                    av_cur = None
                for g in range(len(GROUPS)):
                    if e_cur is not None:
                        emit_qk_group(nt, g, e_cur)
                    if av_cur is not None:
                        emit_av_group(av_cur, e_tiles[nt - 1], g)
                if nt > 0:
                    e_tiles.pop(nt - 1)
                    emit_post(nt - 1, av_cur)

    nc.finalize()  # Bacc.finalize runs the wait-splitting legalization
    return nc


_cached = {}


def _install_trace_hook():
    """The agent image lacks antenv.axon_hooks, so run_bass_kernel_spmd's
    trace path degrades. Recreate the module + NTFF hook locally."""
    import sys, types
    import antenv
    if "antenv.axon_hooks" in sys.modules:
        return
    mod = types.ModuleType("antenv.axon_hooks")
    holder = {"hook": None}
    mod.set_axon_ntff_profile_hook = lambda h: holder.__setitem__("hook", h)
    mod.get_axon_ntff_profile_hook = lambda: holder["hook"]
    sys.modules["antenv.axon_hooks"] = mod
    antenv.axon_hooks = mod
    from trn_agent_boot.trn_boot import _ntff_profile_via_ctypes
    mod.set_axon_ntff_profile_hook(_ntff_profile_via_ctypes("/opt/axon/libaxon_pjrt.so"))
    import concourse.bass_utils as bu
    bu.upload_artifacts = lambda tmpdir: tmpdir


def make_consts(Wq, bq, Wk, Wv, bv, Wp, bp, gn_w, gn_b):
    f32 = np.float32
    gmask = np.zeros((C, G), f32)
    gbcast2 = np.zeros((G, 128), f32)
    for g in range(G):
        gmask[g * 8:(g + 1) * 8, g] = 1.0 / 8.0
        gbcast2[g, g * 8:(g + 1) * 8] = 1.0
        gbcast2[g, C + g * 8:C + (g + 1) * 8] = 1.0
    WqT = np.asarray(Wq, f32).T
    WkT = np.asarray(Wk, f32).T
    WvT = np.asarray(Wv, f32).T
    Wp_ = np.asarray(Wp, f32)
    cf32 = np.zeros((128, 140), f32)
    cf32[:, 0] = np.tile(np.asarray(bq, f32) / 16.0, 2)
    cf32[0:C, 1] = np.asarray(bp, f32) + Wp_ @ np.asarray(bv, f32)
    cf32[:, 2] = np.tile(np.asarray(gn_w, f32), 2)
    cf32[:, 3] = np.tile(np.asarray(gn_b, f32), 2)
    cf32[0:C, 4:12] = gmask
    cf32[0:G, 12:140] = gbcast2
    cb16 = np.zeros((128, 448), f32)
    cb16[:, 0:128] = np.tile(WqT, (2, 2)) / 32.0
    cb16[:, 128:256] = np.tile(WkT, (2, 2)) / 2.0
    cb16[:, 256:320] = np.tile(WvT, (2, 1)) / 2.0
    cb16[0:C, 320:384] = Wp_.T
    cb16[0:C, 384:448] = (Wp_ @ np.asarray(Wv, f32)).T
    return {
        "cf32": np.ascontiguousarray(cf32),
        "cb16": np.ascontiguousarray(cb16.astype(ml_dtypes.bfloat16)),
    }


def kernel(x, gn_w, gn_b, Wq, bq, Wk, bk, Wv, bv, Wp, bp, _trace=False):
    x = np.ascontiguousarray(np.asarray(x, np.float32)).reshape(B, C, N)
    consts = make_consts(Wq, bq, Wk, Wv, bv, Wp, bp, gn_w, gn_b)

    if _trace:
        _install_trace_hook()

    if "nc" not in _cached:
        _cached["nc"] = build_program()
    nc = _cached["nc"]

    in_maps = [dict(consts, x=np.ascontiguousarray(x[i])) for i in range(B)]
    res = run_bass_kernel_spmd(nc, in_maps, core_ids=list(range(B)), trace=_trace)
    last_run_info["exec_time_ns"] = res.exec_time_ns
    last_run_info["mean_exec_time_ns"] = res.mean_exec_time_ns
    out = np.stack([res.results[i]["out"] for i in range(B)], axis=0)
    return out.reshape(B, C, H, W)


# revision 46
# speedup vs baseline: 1.1890x; 1.1890x over previous
"""AttentionBlock (GroupNorm -> 1x1-conv QKV -> softmax attention -> 1x1-conv proj
-> residual) for Trainium2, data-parallel over batch across 8 NeuronCores.

Shapes (hardcoded): x [B=8, C=64, H=64, W=64] fp32; N = H*W = 4096.
Each core processes one sample end-to-end; no cross-core communication.

Key Trainium facts that shape this kernel:
  - A matmul with contraction K<=64 streams at HALF rate (64-row tiling mode);
    K=128 streams 1 column/cycle. With C=64 channels, all hot matmuls are
    made K=128 by duplicating operands on both partition halves and halving
    the stacked weights (sum over 128 partitions of duplicated data = 2x).
  - fp32 matmuls run as two PE passes and their self-loading LDWEIGHTS only
    supports one sync wait; bf16 is one pass (and scores are O(1), so bf16
    keeps ~3 digits -> final error ~1e-4).
  - ScalarE exp runs at 1 elem/lane/cycle -> 16.7M exps/core ~ 115us is the
    roofline engine; everything else is arranged to hide under it.

Per-core pipeline:
  1. GroupNorm: per-channel bn_stats/bn_aggr on x2x[0:64] -> tiny mask
     matmuls reduce/broadcast the 8-channel groups -> one fused affine
     produces h2x [128, N] bf16 (h duplicated on both partition halves).
  2. q2x = (Wq h + bq)/16 and k2x = Wk h, both [128, N] bf16 duplicated
     (bk dropped: constant shift per softmax row). vT [N, C+1] bf16 with a
     ones column so the AV matmul also accumulates the softmax denominator.
  3. sT[m, n] tiles = k2x.T @ q2x (K=128), exp on ScalarE PSUM->SBUF (score
     range is ~[-3, 3]: no row-max subtraction needed), AV accumulates
     out[c, n] + den[n] over the 32 m-chunks.
  4. proj = Wp @ out_unnormalized, scaled by 1/den (column scaling commutes
     with the left matmul; reciprocal via a DMA partition-broadcast of den
     and the fast DVE approx reciprocal), + (bp + Wp bv) + residual x.

The nt loop is software-pipelined (scores/exp of tile nt interleaved with AV
of tile nt-1) so the PE stream stays dense and ScalarE never starves.
"""

import os
import numpy as np
import ml_dtypes

import concourse.bass as bass
import concourse.bacc as bacc
import concourse.mybir as mybir
from concourse.tile import TileContext
from concourse.bass_utils import run_bass_kernel_spmd

FP = mybir.dt.float32
F16 = mybir.dt.bfloat16
B, C, H, W = 8, 64, 64, 64
N = H * W          # 4096
G = 8              # groups
NT = 512           # n-tile (free dim of score tiles)
MT = 128           # m-tile (partition dim of score tiles)
N_NT = N // NT     # 8
N_MT = N // MT     # 32
NPAIR = N_MT // 2  # 16 score psum groups (2 m-chunks each) per n-tile
EPS = 1e-5
COPY = mybir.ActivationFunctionType.Copy

last_run_info = {}


class OneActSetBacc(bacc.Bacc):
    """All ACT functions used here (exp, ln, square, copy) live in the
    natural_log_exp_and_others table set (id 6). The default per-function
    set choice inserts three ~1.3us table loads on the critical path; force
    every load to set 6 and drop the redundant reloads."""

    NL_EXP_SET = 6

    def insert_act_table_loads(self):
        super().insert_act_table_loads()
        for blk in self.main_func.blocks:
            keep = []
            seen = False
            for ins in blk.instructions:
                if isinstance(ins, mybir.InstLoadActFuncSet):
                    ins.act_func_set_id = self.NL_EXP_SET
                    si = ins.sync_info
                    clean = si is None or (not si.on_wait and not si.on_update)
                    if seen and clean:
                        continue
                    seen = True
                keep.append(ins)
            if len(keep) != len(blk.instructions):
                blk.instructions[:] = keep


def build_program(debug=False):
    # Bacc (not raw Bass): its finalize pipeline splits multi-sem waits.
    nc = OneActSetBacc()
    dbg = {}
    if debug:
        for nm, shp in [("dbg_h", [128, N]), ("dbg_q", [128, N]), ("dbg_k", [128, N]),
                        ("dbg_vt", [128, N_MT * (C + 1)]),
                        ("dbg_av", [C, N]), ("dbg_den", [1, N])]:
            dbg[nm] = nc.dram_tensor(nm, shp, FP, kind="ExternalOutput")

    x_d = nc.dram_tensor("x", [C, N], FP, kind="ExternalInput")
    # All small constants packed into two tensors (one DMA each):
    # cf32 [128, 140]: 0 bq2 | 1 bpp | 2 gamma2 | 3 beta2 | 4:12 gmask | 12:140 gbcast2(rows 0:8)
    # cb16 [128, 448]: 0:128 wq_st | 128:256 wk_st | 256:320 wv_st | 320:384 wpT | 384:448 wpwvT
    cf32_d = nc.dram_tensor("cf32", [128, 140], FP, kind="ExternalInput")
    cb16_d = nc.dram_tensor("cb16", [128, 448], F16, kind="ExternalInput")
    out_d = nc.dram_tensor("out", [C, N], FP, kind="ExternalOutput")

    with TileContext(nc) as tc:
        with (
            tc.tile_pool(name="const", bufs=1) as const,
            tc.tile_pool(name="big", bufs=1) as big,
            tc.tile_pool(name="epool", bufs=2) as epool,
            tc.tile_pool(name="small", bufs=4) as small,
            tc.tile_pool(name="outp", bufs=3) as outp,
            tc.tile_pool(name="dram", bufs=2, space="DRAM") as drampool,
            tc.tile_pool(name="qk_ps", bufs=2, space="PSUM") as qk_ps,
            tc.tile_pool(name="av_ps", bufs=2, space="PSUM") as av_ps,
            tc.tile_pool(name="post_ps", bufs=2, space="PSUM") as post_ps,
        ):
            # ---- constant loads (2 packed DMAs; DVE-funneled because a
            # matmul's self-loading LDWEIGHTS supports only one sync wait,
            # so matmul operands must not depend directly on DMA) ----
            cf32s = small.tile([128, 140], FP, tag="cf32s")
            cb16s = small.tile([128, 448], F16, tag="cb16s")
            nc.sync.dma_start(out=cf32s[:], in_=cf32_d[:])
            nc.sync.dma_start(out=cb16s[:], in_=cb16_d[:])
            cf32 = const.tile([128, 140], FP, tag="cf32")
            cb16 = const.tile([128, 448], F16, tag="cb16")
            nc.vector.tensor_copy(out=cf32[:], in_=cf32s[:])
            nc.vector.tensor_copy(out=cb16[:], in_=cb16s[:])
            bq2 = cf32[:, 0:1]
            bpp = cf32[0:C, 1:2]
            gamma2 = cf32[:, 2:3]
            beta2 = cf32[:, 3:4]
            gmask = cf32[0:C, 4:12]
            gbcast2 = cf32[0:G, 12:140]
            wq_st = cb16[:, 0:128]
            wk_st = cb16[:, 128:256]
            wv_st = cb16[:, 256:320]
            wpT = cb16[0:C, 320:384]
            wpwvT = cb16[0:C, 384:448]

            eps_sb = const.tile([128, 1], FP, tag="eps")
            nc.vector.memset(eps_sb[:], EPS)
            ones_col = const.tile([128, C], F16, tag="ones_col")
            nc.vector.memset(ones_col[:], 1.0)

            # ---- load x duplicated on both halves; stats + bf16 cast ----
            # chunked so stats and the x16 cast pipeline with the DMA
            x2x = big.tile([128, N], FP, tag="x2x")
            x16 = big.tile([128, N], F16, tag="x16")
            sums = small.tile([C, 2, 2], FP, tag="gn_sums")
            sq_scr = small.tile([C, N // 2], FP, tag="gn_sq_scr")
            for j in range(2):
                sl = slice(j * (N // 2), (j + 1) * (N // 2))
                nc.sync.dma_start(out=x2x[0:C, sl], in_=x_d[:, sl])
                nc.sync.dma_start(out=x2x[C:128, sl], in_=x_d[:, sl])
                nc.scalar.activation(out=sq_scr[:], in_=x2x[0:C, sl],
                                     func=mybir.ActivationFunctionType.Square,
                                     accum_out=sums[:, j, 1:2])
                nc.vector.tensor_reduce(op=mybir.AluOpType.add, out=sums[:, j, 0:1],
                                        in_=x2x[0:C, sl], axis=mybir.AxisListType.X)
                nc.vector.tensor_copy(out=x16[:, sl], in_=x2x[:, sl])
            mm2 = small.tile([C, 2], FP, tag="gn_mm2")
            nc.vector.tensor_add(out=sums[:, 0, :], in0=sums[:, 0, :], in1=sums[:, 1, :])
            nc.vector.tensor_scalar_mul(out=mm2[:], in0=sums[:, 0, :], scalar1=1.0 / N)
            # group stats: [G, 2] = gmask.T @ mm2   (gmask holds 1/8)
            gstat_ps = post_ps.tile([128, 512], FP, tag="post")
            nc.tensor.matmul(out=gstat_ps[0:G, 0:2], lhsT=gmask, rhs=mm2[:])
            gstat = small.tile([G, 2], FP, tag="gn_gstat")
            nc.vector.tensor_copy(out=gstat[:], in_=gstat_ps[0:G, 0:2])
            # var_g = E[x^2]_g - mean_g^2 ; rstd = 1/sqrt(var+eps)
            vg = small.tile([G, 1], FP, tag="gn_vg")
            nc.vector.tensor_mul(out=vg[:], in0=gstat[:, 0:1], in1=gstat[:, 0:1])
            nc.vector.tensor_sub(out=vg[:], in0=gstat[:, 1:2], in1=vg[:])
            # rstd = exp(-0.5*ln(var+eps)) — Ln and Exp share one ACT table
            # set with the attention exp, avoiding a 2.7us sqrt-table load.
            lnv = small.tile([G, 1], FP, tag="gn_lnv")
            nc.scalar.activation(out=lnv[:], in_=vg[:],
                                 func=mybir.ActivationFunctionType.Ln,
                                 bias=eps_sb[0:G, :])
            rhs2 = small.tile([G, 2], FP, tag="gn_rhs2")
            nc.vector.tensor_copy(out=rhs2[:, 0:1], in_=gstat[:, 0:1])
            nc.scalar.activation(out=rhs2[:, 1:2], in_=lnv[:],
                                 func=mybir.ActivationFunctionType.Exp,
                                 scale=-0.5)
            # broadcast to both channel copies: [128, 2] = gbcast2.T @ rhs2
            pstat_ps = post_ps.tile([128, 512], FP, tag="post")
            nc.tensor.matmul(out=pstat_ps[:, 0:2], lhsT=gbcast2, rhs=rhs2[:])
            a_sb = small.tile([128, 1], FP, tag="gn_a")
            b_sb = small.tile([128, 1], FP, tag="gn_b")
            nc.vector.tensor_mul(out=a_sb[:], in0=pstat_ps[:, 1:2], in1=gamma2[:])
            nc.vector.tensor_mul(out=b_sb[:], in0=pstat_ps[:, 0:1], in1=a_sb[:])
            nc.vector.tensor_sub(out=b_sb[:], in0=beta2[:], in1=b_sb[:])
            # Fold the affine h = a*x + b into the projections:
            #   w*_eff = w*_st * a (per-partition row scale)
            #   q bias += (Wq b)/16 via a tiny matmul; k's b-term shifts every
            #   score in a softmax row by a constant (drop); v's b-term folds
            #   into the final bias as Wp @ Wv @ b (wpwv const, tiny matmul).
            b16 = small.tile([128, 1], F16, tag="gn_b16")
            nc.vector.tensor_copy(out=b16[:], in_=b_sb[:])
            wq_eff = const.tile([128, 128], F16, tag="wq_eff")
            wk_eff = const.tile([128, 128], F16, tag="wk_eff")
            wv_eff = const.tile([128, C], F16, tag="wv_eff")
            nc.vector.tensor_scalar_mul(out=wq_eff[:], in0=wq_st, scalar1=a_sb[:])
            nc.vector.tensor_scalar_mul(out=wk_eff[:], in0=wk_st, scalar1=a_sb[:])
            nc.vector.tensor_scalar_mul(out=wv_eff[:], in0=wv_st, scalar1=a_sb[:])
            bias_ps = post_ps.tile([128, 512], FP, tag="post")
            nc.tensor.matmul(out=bias_ps[:, 0:1], lhsT=wq_st, rhs=b16[:])
            nc.tensor.matmul(out=bias_ps[0:C, 1:2], lhsT=wpwvT, rhs=b16[0:C, :])
            bq_eff = small.tile([128, 1], FP, tag="bq_eff")
            bpp_eff = small.tile([C, 1], FP, tag="bpp_eff")
            nc.vector.tensor_add(out=bq_eff[:], in0=bias_ps[:, 0:1], in1=bq2)
            nc.vector.tensor_add(out=bpp_eff[:], in0=bias_ps[0:C, 1:2], in1=bpp)

            # ---- QKV projections (bf16, K=128) ----
            q2x = big.tile([128, N], F16, tag="q2x")
            k2x = big.tile([128, N], F16, tag="k2x")
            for j in range(N_NT):
                sl = slice(j * NT, (j + 1) * NT)
                qp = qk_ps.tile([128, 2 * NT], FP, tag="qk")
                nc.tensor.matmul(out=qp[:, 0:NT], lhsT=wq_eff[:], rhs=x16[:, sl])
                nc.tensor.matmul(out=qp[:, NT:2 * NT], lhsT=wk_eff[:], rhs=x16[:, sl])
                # q needs a bias add (VectorE); k is a plain copy (ScalarE)
                nc.vector.tensor_scalar_add(out=q2x[:, sl], in0=qp[:, 0:NT], scalar1=bq_eff[:])
                nc.scalar.activation(out=k2x[:, sl], in_=qp[:, NT:2 * NT], func=COPY)

            # vT_aug[p, mt, 0:64] = v[m = mt*128+p, c]; vT_aug[p, mt, 64] = 1
            vT = big.tile([128, N_MT, C + 1], F16, tag="vT")
            nc.vector.memset(vT[:, :, C:C + 1], 1.0)
            for mt in range(0, N_MT, 4):
                vp = av_ps.tile([128, NT], FP, tag="av")
                for j in range(4):
                    nc.tensor.matmul(out=vp[:, j * C:(j + 1) * C],
                                     lhsT=x16[:, (mt + j) * MT:(mt + j + 1) * MT],
                                     rhs=wv_eff[:])
                nc.scalar.activation(
                    out=vT[:, mt:mt + 4, 0:C],
                    in_=vp[:, 0:4 * C].rearrange("p (j c) -> p j c", j=4),
                    func=COPY)

            if debug:
                dq = big.tile([128, N], FP, tag="dbgq")
                dk = big.tile([128, N], FP, tag="dbgk")
                dv = big.tile([128, N_MT * (C + 1)], FP, tag="dbgv")
                nc.vector.tensor_copy(out=dq[:], in_=q2x[:])
                nc.vector.tensor_copy(out=dk[:], in_=k2x[:])
                nc.vector.tensor_copy(out=dv[:], in_=vT[:].rearrange("p a b -> p (a b)"))
                nc.sync.dma_start(out=dbg["dbg_q"][:], in_=dq[:])
                nc.sync.dma_start(out=dbg["dbg_k"][:], in_=dk[:])
                nc.sync.dma_start(out=dbg["dbg_vt"][:], in_=dv[:])

            # ---- attention (software-pipelined over n-tiles) ----
            e_tiles = {}

            # m-chunk grouping per n-tile: 10 groups of 3 + 1 of 2 so each
            # exp instruction covers [128, 1536] (amortizes ScalarE's
            # per-instruction overhead; 3 PSUM banks per group).
            GROUPS = [(i * 2, 2) for i in range(16)]

            def emit_qk_group(nt, g, e):
                nsl = slice(nt * NT, (nt + 1) * NT)
                mt0, gsz = GROUPS[g]
                sp = qk_ps.tile([128, 2 * NT], FP, tag="qk")
                for j in range(gsz):
                    mt = mt0 + j
                    nc.tensor.matmul(out=sp[:, j * NT:(j + 1) * NT],
                                     lhsT=k2x[:, mt * MT:(mt + 1) * MT],
                                     rhs=q2x[:, nsl])
                nc.scalar.activation(out=e[:, mt0:mt0 + gsz, :],
                                     in_=sp[:, 0:gsz * NT],
                                     func=mybir.ActivationFunctionType.Exp)

            def emit_av_group(av, e, g):
                mt0, gsz = GROUPS[g]
                for j in range(gsz):
                    mt = mt0 + j
                    nc.tensor.matmul(
                        out=av[0:C + 1, :],
                        lhsT=vT[:, mt, :],
                        rhs=e[:, mt, :],
                        start=(mt == 0), stop=(mt == N_MT - 1),
                        skip_group_check=True)

            def emit_post(nt, av):
                nsl = slice(nt * NT, (nt + 1) * NT)
                # den (psum row 64) -> SBUF -> partition-broadcast via DMA
                # (DRAM bounce) -> fast approx reciprocal on 64 partitions.
                den16 = small.tile([128, NT], F16, tag="den16")
                nc.vector.tensor_copy(out=den16[C:C + 1, :], in_=av[C:C + 1, :])
                if debug:
                    den_sb = small.tile([128, NT], FP, tag="den_sb")
                    nc.vector.tensor_copy(out=den_sb[C:C + 1, :], in_=av[C:C + 1, :])
                    nc.sync.dma_start(out=dbg["dbg_den"][:, nsl], in_=den_sb[C:C + 1, :])
                dbc_ps = post_ps.tile([128, 512], FP, tag="post")
                nc.tensor.matmul(out=dbc_ps[0:C, :], lhsT=ones_col[C:C + 1, :],
                                 rhs=den16[C:C + 1, :])
                den_bc = outp.tile([C, NT], FP, tag="den_bc")
                nc.vector.tensor_copy(out=den_bc[:], in_=dbc_ps[0:C, :])
                dbc = outp.tile([C, NT], FP, tag="dbc")
                scr = outp.tile([C, NT], FP, tag="dbc_scr")
                nc.vector.reciprocal_approx_accurate(out=dbc[:], in_=den_bc[:], scratch=scr[:])
                # unnormalized attention output -> SBUF (bf16) for proj matmul
                av_sb = outp.tile([C, NT], F16, tag="av_sb")
                nc.vector.tensor_copy(out=av_sb[:], in_=av[0:C, :])
                if debug:
                    dav = outp.tile([C, NT], FP, tag="dav")
                    nc.vector.tensor_copy(out=dav[:], in_=av[0:C, :])
                    nc.sync.dma_start(out=dbg["dbg_av"][:, nsl], in_=dav[:])
                # proj, then scale columns by 1/den, + bias' + residual
                pj_ps = post_ps.tile([128, 512], FP, tag="post")
                nc.tensor.matmul(out=pj_ps[0:C, :], lhsT=wpT, rhs=av_sb[:])
                o_sb = outp.tile([C, NT], FP, tag="o_sb")
                nc.vector.tensor_mul(out=o_sb[:], in0=pj_ps[0:C, :], in1=dbc[:])
                nc.vector.scalar_tensor_tensor(
                    out=o_sb[:], in0=o_sb[:], scalar=bpp_eff[:], in1=x2x[0:C, nsl],
                    op0=mybir.AluOpType.add, op1=mybir.AluOpType.add)
                nc.sync.dma_start(out=out_d[:, nsl], in_=o_sb[:])

            for nt in range(N_NT + 1):
                e_cur = None
                if nt < N_NT:
                    e_cur = epool.tile([128, N_MT, NT], F16, tag="e")
                    e_tiles[nt] = e_cur
                if nt > 0:
                    av_cur = av_ps.tile([128, NT], FP, tag="av", name=f"av_{nt}")
                else:
                    av_cur = None
                for g in range(len(GROUPS)):
                    if e_cur is not None:
                        emit_qk_group(nt, g, e_cur)
                    if av_cur is not None:
                        emit_av_group(av_cur, e_tiles[nt - 1], g)
                if nt > 0:
                    e_tiles.pop(nt - 1)
                    emit_post(nt - 1, av_cur)

    nc.finalize()  # Bacc.finalize runs the wait-splitting legalization
    return nc


_cached = {}


def _install_trace_hook():
    """The agent image lacks antenv.axon_hooks, so run_bass_kernel_spmd's
    trace path degrades. Recreate the module + NTFF hook locally."""
    import sys, types
    import antenv
    if "antenv.axon_hooks" in sys.modules:
        return
    mod = types.ModuleType("antenv.axon_hooks")
    holder = {"hook": None}
    mod.set_axon_ntff_profile_hook = lambda h: holder.__setitem__("hook", h)
    mod.get_axon_ntff_profile_hook = lambda: holder["hook"]
    sys.modules["antenv.axon_hooks"] = mod
    antenv.axon_hooks = mod
    from trn_agent_boot.trn_boot import _ntff_profile_via_ctypes
    mod.set_axon_ntff_profile_hook(_ntff_profile_via_ctypes("/opt/axon/libaxon_pjrt.so"))
    import concourse.bass_utils as bu
    bu.upload_artifacts = lambda tmpdir: tmpdir


def make_consts(Wq, bq, Wk, Wv, bv, Wp, bp, gn_w, gn_b):
    f32 = np.float32
    gmask = np.zeros((C, G), f32)
    gbcast2 = np.zeros((G, 128), f32)
    for g in range(G):
        gmask[g * 8:(g + 1) * 8, g] = 1.0 / 8.0
        gbcast2[g, g * 8:(g + 1) * 8] = 1.0
        gbcast2[g, C + g * 8:C + (g + 1) * 8] = 1.0
    WqT = np.asarray(Wq, f32).T
    WkT = np.asarray(Wk, f32).T
    WvT = np.asarray(Wv, f32).T
    Wp_ = np.asarray(Wp, f32)
    cf32 = np.zeros((128, 140), f32)
    cf32[:, 0] = np.tile(np.asarray(bq, f32) / 16.0, 2)
    cf32[0:C, 1] = np.asarray(bp, f32) + Wp_ @ np.asarray(bv, f32)
    cf32[:, 2] = np.tile(np.asarray(gn_w, f32), 2)
    cf32[:, 3] = np.tile(np.asarray(gn_b, f32), 2)
    cf32[0:C, 4:12] = gmask
    cf32[0:G, 12:140] = gbcast2
    cb16 = np.zeros((128, 448), f32)
    cb16[:, 0:128] = np.tile(WqT, (2, 2)) / 32.0
    cb16[:, 128:256] = np.tile(WkT, (2, 2)) / 2.0
    cb16[:, 256:320] = np.tile(WvT, (2, 1)) / 2.0
    cb16[0:C, 320:384] = Wp_.T
    cb16[0:C, 384:448] = (Wp_ @ np.asarray(Wv, f32)).T
    return {
        "cf32": np.ascontiguousarray(cf32),
        "cb16": np.ascontiguousarray(cb16.astype(ml_dtypes.bfloat16)),
    }


def kernel(x, gn_w, gn_b, Wq, bq, Wk, bk, Wv, bv, Wp, bp, _trace=False):
    x = np.ascontiguousarray(np.asarray(x, np.float32)).reshape(B, C, N)
    consts = make_consts(Wq, bq, Wk, Wv, bv, Wp, bp, gn_w, gn_b)

    if _trace:
        _install_trace_hook()

    if "nc" not in _cached:
        _cached["nc"] = build_program()
    nc = _cached["nc"]

    in_maps = [dict(consts, x=np.ascontiguousarray(x[i])) for i in range(B)]
    res = run_bass_kernel_spmd(nc, in_maps, core_ids=list(range(B)), trace=_trace)
    last_run_info["exec_time_ns"] = res.exec_time_ns
    last_run_info["mean_exec_time_ns"] = res.mean_exec_time_ns
    out = np.stack([res.results[i]["out"] for i in range(B)], axis=0)
    return out.reshape(B, C, H, W)


# revision 47
# speedup vs baseline: 1.2093x; 1.0170x over previous
"""AttentionBlock (GroupNorm -> 1x1-conv QKV -> softmax attention -> 1x1-conv proj
-> residual) for Trainium2, data-parallel over batch across 8 NeuronCores.

Shapes (hardcoded): x [B=8, C=64, H=64, W=64] fp32; N = H*W = 4096.
Each core processes one sample end-to-end; no cross-core communication.

Key Trainium facts that shape this kernel:
  - A matmul with contraction K<=64 streams at HALF rate (64-row tiling mode);
    K=128 streams 1 column/cycle. With C=64 channels, all hot matmuls are
    made K=128 by duplicating operands on both partition halves and halving
    the stacked weights (sum over 128 partitions of duplicated data = 2x).
  - fp32 matmuls run as two PE passes and their self-loading LDWEIGHTS only
    supports one sync wait; bf16 is one pass (and scores are O(1), so bf16
    keeps ~3 digits -> final error ~1e-4).
  - ScalarE exp runs at 1 elem/lane/cycle -> 16.7M exps/core ~ 115us is the
    roofline engine; everything else is arranged to hide under it.

Per-core pipeline:
  1. GroupNorm: per-channel bn_stats/bn_aggr on x2x[0:64] -> tiny mask
     matmuls reduce/broadcast the 8-channel groups -> one fused affine
     produces h2x [128, N] bf16 (h duplicated on both partition halves).
  2. q2x = (Wq h + bq)/16 and k2x = Wk h, both [128, N] bf16 duplicated
     (bk dropped: constant shift per softmax row). vT [N, C+1] bf16 with a
     ones column so the AV matmul also accumulates the softmax denominator.
  3. sT[m, n] tiles = k2x.T @ q2x (K=128), exp on ScalarE PSUM->SBUF (score
     range is ~[-3, 3]: no row-max subtraction needed), AV accumulates
     out[c, n] + den[n] over the 32 m-chunks.
  4. proj = Wp @ out_unnormalized, scaled by 1/den (column scaling commutes
     with the left matmul; reciprocal via a DMA partition-broadcast of den
     and the fast DVE approx reciprocal), + (bp + Wp bv) + residual x.

The nt loop is software-pipelined (scores/exp of tile nt interleaved with AV
of tile nt-1) so the PE stream stays dense and ScalarE never starves.
"""

import os
import numpy as np
import ml_dtypes

import concourse.bass as bass
import concourse.bacc as bacc
import concourse.mybir as mybir
from concourse.tile import TileContext
from concourse.bass_utils import run_bass_kernel_spmd

FP = mybir.dt.float32
F16 = mybir.dt.bfloat16
B, C, H, W = 8, 64, 64, 64
N = H * W          # 4096
G = 8              # groups
NT = 512           # n-tile (free dim of score tiles)
MT = 128           # m-tile (partition dim of score tiles)
N_NT = N // NT     # 8
N_MT = N // MT     # 32
NPAIR = N_MT // 2  # 16 score psum groups (2 m-chunks each) per n-tile
EPS = 1e-5
COPY = mybir.ActivationFunctionType.Copy

last_run_info = {}


class OneActSetBacc(bacc.Bacc):
    """All ACT functions used here (exp, ln, square, copy) live in the
    natural_log_exp_and_others table set (id 6). The default per-function
    set choice inserts three ~1.3us table loads on the critical path; force
    every load to set 6 and drop the redundant reloads."""

    NL_EXP_SET = 6

    def insert_act_table_loads(self):
        super().insert_act_table_loads()
        for blk in self.main_func.blocks:
            keep = []
            seen = False
            for ins in blk.instructions:
                if isinstance(ins, mybir.InstLoadActFuncSet):
                    ins.act_func_set_id = self.NL_EXP_SET
                    si = ins.sync_info
                    clean = si is None or (not si.on_wait and not si.on_update)
                    if seen and clean:
                        continue
                    seen = True
                keep.append(ins)
            if len(keep) != len(blk.instructions):
                blk.instructions[:] = keep


def build_program(debug=False):
    # Bacc (not raw Bass): its finalize pipeline splits multi-sem waits.
    nc = OneActSetBacc()
    dbg = {}
    if debug:
        for nm, shp in [("dbg_h", [128, N]), ("dbg_q", [128, N]), ("dbg_k", [128, N]),
                        ("dbg_vt", [128, N_MT * (C + 1)]),
                        ("dbg_av", [C, N]), ("dbg_den", [1, N])]:
            dbg[nm] = nc.dram_tensor(nm, shp, FP, kind="ExternalOutput")

    x_d = nc.dram_tensor("x", [C, N], FP, kind="ExternalInput")
    # All small constants packed into two tensors (one DMA each):
    # cf32 [128, 140]: 0 bq2 | 1 bpp | 2 gamma2 | 3 beta2 | 4:12 gmask | 12:140 gbcast2(rows 0:8)
    # cb16 [128, 448]: 0:128 wq_st | 128:256 wk_st | 256:320 wv_st | 320:384 wpT | 384:448 wpwvT
    cf32_d = nc.dram_tensor("cf32", [128, 140], FP, kind="ExternalInput")
    cb16_d = nc.dram_tensor("cb16", [128, 448], F16, kind="ExternalInput")
    out_d = nc.dram_tensor("out", [C, N], FP, kind="ExternalOutput")

    with TileContext(nc) as tc:
        with (
            tc.tile_pool(name="const", bufs=1) as const,
            tc.tile_pool(name="big", bufs=1) as big,
            tc.tile_pool(name="epool", bufs=2) as epool,
            tc.tile_pool(name="small", bufs=4) as small,
            tc.tile_pool(name="outp", bufs=3) as outp,
            tc.tile_pool(name="dram", bufs=2, space="DRAM") as drampool,
            tc.tile_pool(name="qk_ps", bufs=2, space="PSUM") as qk_ps,
            tc.tile_pool(name="av_ps", bufs=1, space="PSUM") as av_ps,
            tc.tile_pool(name="post_ps", bufs=1, space="PSUM") as post_ps,
        ):
            # ---- constant loads (2 packed DMAs; DVE-funneled because a
            # matmul's self-loading LDWEIGHTS supports only one sync wait,
            # so matmul operands must not depend directly on DMA) ----
            cf32s = small.tile([128, 140], FP, tag="cf32s")
            cb16s = small.tile([128, 448], F16, tag="cb16s")
            nc.sync.dma_start(out=cf32s[:], in_=cf32_d[:])
            nc.sync.dma_start(out=cb16s[:], in_=cb16_d[:])
            cf32 = const.tile([128, 140], FP, tag="cf32")
            cb16 = const.tile([128, 448], F16, tag="cb16")
            nc.vector.tensor_copy(out=cf32[:], in_=cf32s[:])
            nc.vector.tensor_copy(out=cb16[:], in_=cb16s[:])
            bq2 = cf32[:, 0:1]
            bpp = cf32[0:C, 1:2]
            gamma2 = cf32[:, 2:3]
            beta2 = cf32[:, 3:4]
            gmask = cf32[0:C, 4:12]
            gbcast2 = cf32[0:G, 12:140]
            wq_st = cb16[:, 0:128]
            wk_st = cb16[:, 128:256]
            wv_st = cb16[:, 256:320]
            wpT = cb16[0:C, 320:384]
            wpwvT = cb16[0:C, 384:448]

            eps_sb = const.tile([128, 1], FP, tag="eps")
            nc.vector.memset(eps_sb[:], EPS)
            ones_col = const.tile([128, C], F16, tag="ones_col")
            nc.vector.memset(ones_col[:], 1.0)

            # ---- load x duplicated on both halves; stats + bf16 cast ----
            # chunked so stats and the x16 cast pipeline with the DMA
            x2x = big.tile([128, N], FP, tag="x2x")
            x16 = big.tile([128, N], F16, tag="x16")
            sums = small.tile([C, 2, 2], FP, tag="gn_sums")
            sq_scr = small.tile([C, N // 2], FP, tag="gn_sq_scr")
            for j in range(2):
                sl = slice(j * (N // 2), (j + 1) * (N // 2))
                nc.sync.dma_start(out=x2x[0:C, sl], in_=x_d[:, sl])
                nc.sync.dma_start(out=x2x[C:128, sl], in_=x_d[:, sl])
                nc.scalar.activation(out=sq_scr[:], in_=x2x[0:C, sl],
                                     func=mybir.ActivationFunctionType.Square,
                                     accum_out=sums[:, j, 1:2])
                nc.vector.tensor_reduce(op=mybir.AluOpType.add, out=sums[:, j, 0:1],
                                        in_=x2x[0:C, sl], axis=mybir.AxisListType.X)
                nc.vector.tensor_copy(out=x16[:, sl], in_=x2x[:, sl])
            mm2 = small.tile([C, 2], FP, tag="gn_mm2")
            nc.vector.tensor_add(out=sums[:, 0, :], in0=sums[:, 0, :], in1=sums[:, 1, :])
            nc.vector.tensor_scalar_mul(out=mm2[:], in0=sums[:, 0, :], scalar1=1.0 / N)
            # group stats: [G, 2] = gmask.T @ mm2   (gmask holds 1/8)
            gstat_ps = post_ps.tile([128, 512], FP, tag="post")
            nc.tensor.matmul(out=gstat_ps[0:G, 0:2], lhsT=gmask, rhs=mm2[:])
            gstat = small.tile([G, 2], FP, tag="gn_gstat")
            nc.vector.tensor_copy(out=gstat[:], in_=gstat_ps[0:G, 0:2])
            # var_g = E[x^2]_g - mean_g^2 ; rstd = 1/sqrt(var+eps)
            vg = small.tile([G, 1], FP, tag="gn_vg")
            nc.vector.tensor_mul(out=vg[:], in0=gstat[:, 0:1], in1=gstat[:, 0:1])
            nc.vector.tensor_sub(out=vg[:], in0=gstat[:, 1:2], in1=vg[:])
            # rstd = exp(-0.5*ln(var+eps)) — Ln and Exp share one ACT table
            # set with the attention exp, avoiding a 2.7us sqrt-table load.
            lnv = small.tile([G, 1], FP, tag="gn_lnv")
            nc.scalar.activation(out=lnv[:], in_=vg[:],
                                 func=mybir.ActivationFunctionType.Ln,
                                 bias=eps_sb[0:G, :])
            rhs2 = small.tile([G, 2], FP, tag="gn_rhs2")
            nc.vector.tensor_copy(out=rhs2[:, 0:1], in_=gstat[:, 0:1])
            nc.scalar.activation(out=rhs2[:, 1:2], in_=lnv[:],
                                 func=mybir.ActivationFunctionType.Exp,
                                 scale=-0.5)
            # broadcast to both channel copies: [128, 2] = gbcast2.T @ rhs2
            pstat_ps = post_ps.tile([128, 512], FP, tag="post")
            nc.tensor.matmul(out=pstat_ps[:, 0:2], lhsT=gbcast2, rhs=rhs2[:])
            a_sb = small.tile([128, 1], FP, tag="gn_a")
            b_sb = small.tile([128, 1], FP, tag="gn_b")
            nc.vector.tensor_mul(out=a_sb[:], in0=pstat_ps[:, 1:2], in1=gamma2[:])
            nc.vector.tensor_mul(out=b_sb[:], in0=pstat_ps[:, 0:1], in1=a_sb[:])
            nc.vector.tensor_sub(out=b_sb[:], in0=beta2[:], in1=b_sb[:])
            # Fold the affine h = a*x + b into the projections:
            #   w*_eff = w*_st * a (per-partition row scale)
            #   q bias += (Wq b)/16 via a tiny matmul; k's b-term shifts every
            #   score in a softmax row by a constant (drop); v's b-term folds
            #   into the final bias as Wp @ Wv @ b (wpwv const, tiny matmul).
            b16 = small.tile([128, 1], F16, tag="gn_b16")
            nc.vector.tensor_copy(out=b16[:], in_=b_sb[:])
            wq_eff = const.tile([128, 128], F16, tag="wq_eff")
            wk_eff = const.tile([128, 128], F16, tag="wk_eff")
            wv_eff = const.tile([128, C], F16, tag="wv_eff")
            nc.vector.tensor_scalar_mul(out=wq_eff[:], in0=wq_st, scalar1=a_sb[:])
            nc.vector.tensor_scalar_mul(out=wk_eff[:], in0=wk_st, scalar1=a_sb[:])
            nc.vector.tensor_scalar_mul(out=wv_eff[:], in0=wv_st, scalar1=a_sb[:])
            bias_ps = post_ps.tile([128, 512], FP, tag="post")
            nc.tensor.matmul(out=bias_ps[:, 0:1], lhsT=wq_st, rhs=b16[:])
            nc.tensor.matmul(out=bias_ps[0:C, 1:2], lhsT=wpwvT, rhs=b16[0:C, :])
            bq_eff = small.tile([128, 1], FP, tag="bq_eff")
            bpp_eff = small.tile([C, 1], FP, tag="bpp_eff")
            nc.vector.tensor_add(out=bq_eff[:], in0=bias_ps[:, 0:1], in1=bq2)
            nc.vector.tensor_add(out=bpp_eff[:], in0=bias_ps[0:C, 1:2], in1=bpp)

            # ---- QKV projections (bf16, K=128) ----
            q2x = big.tile([128, N], F16, tag="q2x")
            k2x = big.tile([128, N], F16, tag="k2x")
            for j in range(N_NT):
                sl = slice(j * NT, (j + 1) * NT)
                qp = qk_ps.tile([128, 2 * NT], FP, tag="qk")
                nc.tensor.matmul(out=qp[:, 0:NT], lhsT=wq_eff[:], rhs=x16[:, sl])
                nc.tensor.matmul(out=qp[:, NT:2 * NT], lhsT=wk_eff[:], rhs=x16[:, sl])
                # q needs a bias add (VectorE); k is a plain copy (ScalarE)
                nc.vector.tensor_scalar_add(out=q2x[:, sl], in0=qp[:, 0:NT], scalar1=bq_eff[:])
                nc.scalar.activation(out=k2x[:, sl], in_=qp[:, NT:2 * NT], func=COPY)

            # vT_aug[p, mt, 0:64] = v[m = mt*128+p, c]; vT_aug[p, mt, 64] = 1
            vT = big.tile([128, N_MT, C + 1], F16, tag="vT")
            nc.vector.memset(vT[:, :, C:C + 1], 1.0)
            for mt in range(0, N_MT, 4):
                vp = av_ps.tile([128, NT], FP, tag="av")
                for j in range(4):
                    nc.tensor.matmul(out=vp[:, j * C:(j + 1) * C],
                                     lhsT=x16[:, (mt + j) * MT:(mt + j + 1) * MT],
                                     rhs=wv_eff[:])
                nc.scalar.activation(
                    out=vT[:, mt:mt + 4, 0:C],
                    in_=vp[:, 0:4 * C].rearrange("p (j c) -> p j c", j=4),
                    func=COPY)

            if debug:
                dq = big.tile([128, N], FP, tag="dbgq")
                dk = big.tile([128, N], FP, tag="dbgk")
                dv = big.tile([128, N_MT * (C + 1)], FP, tag="dbgv")
                nc.vector.tensor_copy(out=dq[:], in_=q2x[:])
                nc.vector.tensor_copy(out=dk[:], in_=k2x[:])
                nc.vector.tensor_copy(out=dv[:], in_=vT[:].rearrange("p a b -> p (a b)"))
                nc.sync.dma_start(out=dbg["dbg_q"][:], in_=dq[:])
                nc.sync.dma_start(out=dbg["dbg_k"][:], in_=dk[:])
                nc.sync.dma_start(out=dbg["dbg_vt"][:], in_=dv[:])

            # ---- attention (software-pipelined over n-tiles) ----
            e_tiles = {}

            # m-chunk grouping per n-tile: 10 groups of 3 + 1 of 2 so each
            # exp instruction covers [128, 1536] (amortizes ScalarE's
            # per-instruction overhead; 3 PSUM banks per group).
            GROUPS = [(i * 3, 3) for i in range(10)] + [(30, 2)]

            def emit_qk_group(nt, g, e):
                nsl = slice(nt * NT, (nt + 1) * NT)
                mt0, gsz = GROUPS[g]
                sp = qk_ps.tile([128, 3 * NT], FP, tag="qk")
                for j in range(gsz):
                    mt = mt0 + j
                    nc.tensor.matmul(out=sp[:, j * NT:(j + 1) * NT],
                                     lhsT=k2x[:, mt * MT:(mt + 1) * MT],
                                     rhs=q2x[:, nsl])
                nc.scalar.activation(out=e[:, mt0:mt0 + gsz, :],
                                     in_=sp[:, 0:gsz * NT],
                                     func=mybir.ActivationFunctionType.Exp)

            def emit_av_group(av, e, g):
                mt0, gsz = GROUPS[g]
                for j in range(gsz):
                    mt = mt0 + j
                    nc.tensor.matmul(
                        out=av[0:C + 1, :],
                        lhsT=vT[:, mt, :],
                        rhs=e[:, mt, :],
                        start=(mt == 0), stop=(mt == N_MT - 1),
                        skip_group_check=True)

            def emit_post(nt, av):
                nsl = slice(nt * NT, (nt + 1) * NT)
                # den (psum row 64) -> SBUF -> partition-broadcast via DMA
                # (DRAM bounce) -> fast approx reciprocal on 64 partitions.
                den16 = small.tile([128, NT], F16, tag="den16")
                nc.vector.tensor_copy(out=den16[C:C + 1, :], in_=av[C:C + 1, :])
                if debug:
                    den_sb = small.tile([128, NT], FP, tag="den_sb")
                    nc.vector.tensor_copy(out=den_sb[C:C + 1, :], in_=av[C:C + 1, :])
                    nc.sync.dma_start(out=dbg["dbg_den"][:, nsl], in_=den_sb[C:C + 1, :])
                dbc_ps = post_ps.tile([128, 512], FP, tag="post")
                nc.tensor.matmul(out=dbc_ps[0:C, :], lhsT=ones_col[C:C + 1, :],
                                 rhs=den16[C:C + 1, :])
                den_bc = outp.tile([C, NT], FP, tag="den_bc")
                nc.vector.tensor_copy(out=den_bc[:], in_=dbc_ps[0:C, :])
                dbc = outp.tile([C, NT], FP, tag="dbc")
                scr = outp.tile([C, NT], FP, tag="dbc_scr")
                nc.vector.reciprocal_approx_accurate(out=dbc[:], in_=den_bc[:], scratch=scr[:])
                # unnormalized attention output -> SBUF (bf16) for proj matmul
                av_sb = outp.tile([C, NT], F16, tag="av_sb")
                nc.vector.tensor_copy(out=av_sb[:], in_=av[0:C, :])
                if debug:
                    dav = outp.tile([C, NT], FP, tag="dav")
                    nc.vector.tensor_copy(out=dav[:], in_=av[0:C, :])
                    nc.sync.dma_start(out=dbg["dbg_av"][:, nsl], in_=dav[:])
                # proj, then scale columns by 1/den, + bias' + residual
                pj_ps = post_ps.tile([128, 512], FP, tag="post")
                nc.tensor.matmul(out=pj_ps[0:C, :], lhsT=wpT, rhs=av_sb[:])
                o_sb = outp.tile([C, NT], FP, tag="o_sb")
                nc.vector.tensor_mul(out=o_sb[:], in0=pj_ps[0:C, :], in1=dbc[:])
                nc.vector.scalar_tensor_tensor(
                    out=o_sb[:], in0=o_sb[:], scalar=bpp_eff[:], in1=x2x[0:C, nsl],
                    op0=mybir.AluOpType.add, op1=mybir.AluOpType.add)
                nc.sync.dma_start(out=out_d[:, nsl], in_=o_sb[:])

            for nt in range(N_NT + 1):
                e_cur = None
                if nt < N_NT:
                    e_cur = epool.tile([128, N_MT, NT], F16, tag="e")
                    e_tiles[nt] = e_cur
                if nt > 0:
                    av_cur = av_ps.tile([128, NT], FP, tag="av", name=f"av_{nt}")
                else:
                    av_cur = None
                for g in range(len(GROUPS)):
                    if e_cur is not None:
                        emit_qk_group(nt, g, e_cur)
                    if av_cur is not None:
                        emit_av_group(av_cur, e_tiles[nt - 1], g)
                if nt > 0:
                    e_tiles.pop(nt - 1)
                    emit_post(nt - 1, av_cur)

    nc.finalize()  # Bacc.finalize runs the wait-splitting legalization
    return nc


_cached = {}


def _install_trace_hook():
    """The agent image lacks antenv.axon_hooks, so run_bass_kernel_spmd's
    trace path degrades. Recreate the module + NTFF hook locally."""
    import sys, types
    import antenv
    if "antenv.axon_hooks" in sys.modules:
        return
    mod = types.ModuleType("antenv.axon_hooks")
    holder = {"hook": None}
    mod.set_axon_ntff_profile_hook = lambda h: holder.__setitem__("hook", h)
    mod.get_axon_ntff_profile_hook = lambda: holder["hook"]
    sys.modules["antenv.axon_hooks"] = mod
    antenv.axon_hooks = mod
    from trn_agent_boot.trn_boot import _ntff_profile_via_ctypes
    mod.set_axon_ntff_profile_hook(_ntff_profile_via_ctypes("/opt/axon/libaxon_pjrt.so"))
    import concourse.bass_utils as bu
    bu.upload_artifacts = lambda tmpdir: tmpdir


def make_consts(Wq, bq, Wk, Wv, bv, Wp, bp, gn_w, gn_b):
    f32 = np.float32
    gmask = np.zeros((C, G), f32)
    gbcast2 = np.zeros((G, 128), f32)
    for g in range(G):
        gmask[g * 8:(g + 1) * 8, g] = 1.0 / 8.0
        gbcast2[g, g * 8:(g + 1) * 8] = 1.0
        gbcast2[g, C + g * 8:C + (g + 1) * 8] = 1.0
    WqT = np.asarray(Wq, f32).T
    WkT = np.asarray(Wk, f32).T
    WvT = np.asarray(Wv, f32).T
    Wp_ = np.asarray(Wp, f32)
    cf32 = np.zeros((128, 140), f32)
    cf32[:, 0] = np.tile(np.asarray(bq, f32) / 16.0, 2)
    cf32[0:C, 1] = np.asarray(bp, f32) + Wp_ @ np.asarray(bv, f32)
    cf32[:, 2] = np.tile(np.asarray(gn_w, f32), 2)
    cf32[:, 3] = np.tile(np.asarray(gn_b, f32), 2)
    cf32[0:C, 4:12] = gmask
    cf32[0:G, 12:140] = gbcast2
    cb16 = np.zeros((128, 448), f32)
    cb16[:, 0:128] = np.tile(WqT, (2, 2)) / 32.0
    cb16[:, 128:256] = np.tile(WkT, (2, 2)) / 2.0
    cb16[:, 256:320] = np.tile(WvT, (2, 1)) / 2.0
    cb16[0:C, 320:384] = Wp_.T
    cb16[0:C, 384:448] = (Wp_ @ np.asarray(Wv, f32)).T
    return {
        "cf32": np.ascontiguousarray(cf32),
        "cb16": np.ascontiguousarray(cb16.astype(ml_dtypes.bfloat16)),
    }


def kernel(x, gn_w, gn_b, Wq, bq, Wk, bk, Wv, bv, Wp, bp, _trace=False):
    x = np.ascontiguousarray(np.asarray(x, np.float32)).reshape(B, C, N)
    consts = make_consts(Wq, bq, Wk, Wv, bv, Wp, bp, gn_w, gn_b)

    if _trace:
        _install_trace_hook()

    if "nc" not in _cached:
        _cached["nc"] = build_program()
    nc = _cached["nc"]

    in_maps = [dict(consts, x=np.ascontiguousarray(x[i])) for i in range(B)]
    res = run_bass_kernel_spmd(nc, in_maps, core_ids=list(range(B)), trace=_trace)
    last_run_info["exec_time_ns"] = res.exec_time_ns
    last_run_info["mean_exec_time_ns"] = res.mean_exec_time_ns
    out = np.stack([res.results[i]["out"] for i in range(B)], axis=0)
    return out.reshape(B, C, H, W)
